# revision 28
# baseline (speedup 1.0000x reference)
import sys

if "/opt/trn_rl_repo" not in sys.path:
    sys.path.insert(0, "/opt/trn_rl_repo")

import numpy as np
import ml_dtypes

import concourse.bass as bass
import concourse.tile as tile
from concourse import bacc, mybir
from concourse.bass_utils import run_bass_kernel_spmd

T, N, C, A = 32, 64, 512, 32
F1, F2, F3 = 2048, 1024, 512
NC_ = 8          # neuron cores
NB = N // NC_    # batch per core = 8
FREE = NB * T    # 256 free columns, n-major: index = n*T + t

BF16 = ml_dtypes.bfloat16

_CACHE = {}

# Numerics: spikes are encoded as s' = 2*s - 1 in {-1, +1} so the Scalar
# engine can produce them with a single Sign op per step. Weights are halved
# (W/2) and split hi+lo in bf16 (binary rhs is exact, so the only quantization
# is the ~16-bit weight representation); the affine correction
# W @ s = (W/2) @ s' + rowsum(W/2) is folded in as a K=1 bias matmul into the
# same PSUM accumulation group. The synapse filter is applied after the
# matmul (it commutes with the linear map) as a PSUM->SBUF scan.


def _build(b_eff: float):
    nc = bacc.Bacc("TRN2", target_bir_lowering=False, debug=False, num_devices=NC_)
    f32 = mybir.dt.float32
    bf16 = mybir.dt.bfloat16
    AF = mybir.ActivationFunctionType

    s1T = nc.declare_dram_parameter("s1T", [C, FREE], bf16, isOutput=False)
    wp_par = {}
    for nm, (kd, md) in (("w1", (C, F1)), ("w2", (F1, F2)), ("w3", (F2, F3))):
        wp_par[nm + "h"] = nc.declare_dram_parameter(nm + "h", [kd, md], bf16, isOutput=False)
        wp_par[nm + "l"] = nc.declare_dram_parameter(nm + "l", [kd, md], bf16, isOutput=False)
    wot = nc.declare_dram_parameter("wot", [F3, 1], bf16, isOutput=False)
    rbias = nc.declare_dram_parameter("rbias", [128, 28], f32, isOutput=False)
    maskT = nc.declare_dram_parameter("maskT", [128, 3, FREE], f32, isOutput=False)
    cmaskT = nc.declare_dram_parameter("cmaskT", [1, FREE], f32, isOutput=False)
    out = nc.declare_dram_parameter("out", [1, FREE], f32, isOutput=True)

    AL = mybir.AluOpType

    with tile.TileContext(nc) as tc:
        with (
            tc.tile_pool(name="weights", bufs=1) as wp,
            tc.tile_pool(name="acts", bufs=1) as ap_,
            tc.tile_pool(name="psum", bufs=1, space="PSUM") as pp,
        ):
            # ---- SBUF tiles ----
            m = wp.tile([128, 3, FREE], f32)
            cm = wp.tile([1, FREE], f32)
            rb = wp.tile([128, 28], f32)
            s1 = ap_.tile([128, 4, FREE], bf16)
            w1h = wp.tile([128, 4, F1], bf16)
            w1l = wp.tile([128, 4, F1], bf16)
            w2h = wp.tile([128, 16, F2], bf16)
            w2l = wp.tile([128, 16, F2], bf16)
            w3h = wp.tile([128, 8, F3], bf16)
            w3l = wp.tile([128, 8, F3], bf16)
            wo = wp.tile([128, 4, 1], bf16)

            # filtered pre-activations (become u = v_prev + h in place in IF)
            h2 = ap_.tile([128, 16, NB, T], f32)
            h3 = ap_.tile([128, 8, NB, T], f32)
            h4 = ap_.tile([128, 4, NB, T], f32)
            # sign spikes in {-1,+1} (matmul rhs of the next layer)
            s2 = ap_.tile([128, 16, NB, T], bf16)
            s3 = ap_.tile([128, 8, NB, T], bf16)
            s4 = ap_.tile([128, 4, NB, T], bf16)
            # membranes + dense u scratch (2-buffer rotation)
            v2 = ap_.tile([128, 16, NB], f32)
            v3 = ap_.tile([128, 8, NB], f32)
            v4 = ap_.tile([128, 4, NB], f32)
            ud2 = ap_.tile([128, 4, 16, NB], f32)
            ud3 = ap_.tile([128, 4, 8, NB], f32)
            ud4 = ap_.tile([128, 4, 4, NB], f32)
            wu = ap_.tile([128, 128], bf16)
            wr = ap_.tile([128, FREE], bf16)
            pre = ap_.tile([1, FREE], f32)
            acc = ap_.tile([1, FREE], f32)

            # one accumulator per PSUM bank (start=True clears a whole bank)
            ps = [
                pp.tile([128, FREE], f32, tag=f"ps{i}", name=f"ps{i}")
                for i in range(4)
            ]
            pob = pp.tile([1, 2, FREE], f32, tag="pob")
            pso = pob[:, 0, :]
            psd = pp.tile([128, FREE], f32, tag="psd")

            negone = wp.tile([128, 1], f32)
            nc.vector.memset(negone[:, :], -1.0)
            nc.vector.memset(wu[:, :], 0.0)
            nc.vector.memset(wr[:, :], 0.0)
            # HAM warm-up burst: dense bf16 matmuls on scratch while the
            # weight DMAs stream in, so the real matmuls start at 2.4 GHz
            for i in range(70):
                nc.tensor.matmul(psd[:, :], wu[:, :], wr[:, :], start=True, stop=True)

            # ---- DMAs split across both HWDGE queues (sync + scalar) ----
            def load_w(eng, dst, param, kts, k0=0):
                r = param.ap().rearrange("(kt p) m -> kt p m", p=128)
                for kt in range(k0, kts):
                    eng.dma_start(out=dst[:, kt, :], in_=r[kt])

            # sync queue: inputs + hi weights; scalar queue: only what the ACT
            # engine's own phase needs early (it must drain before Sign ops);
            # gpsimd SWDGE queue: the late lo weights
            nc.sync.dma_start(out=m[:, :, :], in_=maskT.ap())
            nc.sync.dma_start(out=cm[:, :], in_=cmaskT.ap())
            s1r = s1T.ap().rearrange("(kt p) m -> kt p m", p=128)
            for kt in range(4):
                nc.sync.dma_start(out=s1[:, kt, :], in_=s1r[kt])
            nc.scalar.dma_start(out=rb[:, :], in_=rbias.ap())
            # w1 halves split across the two hwdge queues so block 1 starts early
            load_w(nc.sync, w1h, wp_par["w1h"], 2)
            load_w(nc.scalar, w1h, wp_par["w1h"], 4, k0=2)
            load_w(nc.sync, w1l, wp_par["w1l"], 2)
            load_w(nc.scalar, w1l, wp_par["w1l"], 4, k0=2)
            load_w(nc.scalar, wo, wot, 4)
            load_w(nc.sync, w2h, wp_par["w2h"], 16)
            load_w(nc.gpsimd, w2l, wp_par["w2l"], 16)
            load_w(nc.sync, w3h, wp_par["w3h"], 8)
            load_w(nc.gpsimd, w3l, wp_par["w3l"], 8)

            def linear_filtered(h_dst, wh, wl, src, kts, mts, li, roff):
                # h_dst[:, mi] = synapse_filter(W @ spikes + rowsum)[mi];
                # the rowsum correction is a per-partition add on PSUM (DVE)
                for mi in range(mts):
                    b = ps[mi % 4]
                    for kt in range(kts):
                        srck = src[:, kt]
                        if len(srck.shape) == 3:
                            srck = srck.rearrange("p n t -> p (n t)")
                        nc.tensor.matmul(
                            b[:, :], wh[:, kt, bass.ts(mi, 128)], srck,
                            start=(kt == 0), stop=False,
                        )
                        nc.tensor.matmul(
                            b[:, :], wl[:, kt, bass.ts(mi, 128)], srck,
                            start=False, stop=(kt == kts - 1),
                        )
                    nc.vector.tensor_scalar(
                        b[:, :], b[:, :], rb[:, roff + mi: roff + mi + 1],
                        None, AL.add,
                    )
                    nc.vector.tensor_tensor_scan(
                        out=h_dst[:, mi].rearrange("p n t -> p (n t)"),
                        data0=m[:, li, :],
                        data1=b[:, :],
                        initial=0.0, op0=AL.mult, op1=AL.add,
                    )

            def if_layer(h, s, v, ud):
                # u = v_prev + h (dense scratch); spike' = Sign(u - 1) on the
                # Scalar engine (bf16, {-1,+1}); v = u * (u < 1) on DVE.
                nc.vector.memset(v[:, :, :], 0.0)
                for t in range(T):
                    u = ud[:, t % 4]
                    nc.vector.tensor_tensor(u, h[:, :, :, t], v[:, :, :], AL.add)
                    nc.scalar.activation(s[:, :, :, t], u, AF.Sign, bias=negone[:, :])
                    nc.vector.scalar_tensor_tensor(
                        v[:, :, :], u, 1.0, u, AL.is_lt, AL.mult
                    )
                    if t == T - 6:
                        # timed pre-warm burst: first MM is gated on this
                        # step's spikes, the rest follow in PE FIFO order, so
                        # ~3.4us of dense PE work lands right at the IF tail
                        # and HAM is warm when the next layer's matmuls start
                        nc.tensor.matmul(
                            psd[0:1, 0:1], wo[:, 0, :], s[:, 0, 0, t:t + 1],
                            start=True, stop=True,
                        )
                        for _ in range(15):
                            nc.tensor.matmul(
                                psd[:, :], wu[:, :], wr[:, :],
                                start=True, stop=True,
                            )

            # ---- blocks ----
            linear_filtered(h2, w1h, w1l, s1, 4, 16, 0, 0)
            if_layer(h2, s2, v2, ud2)
            linear_filtered(h3, w2h, w2l, s2, 16, 8, 1, 16)
            if_layer(h3, s3, v3, ud3)
            linear_filtered(h4, w3h, w3l, s3, 8, 4, 2, 24)
            if_layer(h4, s4, v4, ud4)

            # ---- head: (W_out/2) @ s4' + b_eff, cumsum over t ----
            for kt in range(4):
                rhs = s4[:, kt].rearrange("p n t -> p (n t)")
                nc.tensor.matmul(
                    pso[:, :], wo[:, kt, :], rhs,
                    start=(kt == 0), stop=(kt == 3),
                )
            nc.vector.tensor_scalar_add(pre[:, :], pso[:, :], float(b_eff))
            nc.vector.tensor_tensor_scan(
                out=acc[:, :], data0=cm[:, :], data1=pre[:, :],
                initial=0.0, op0=AL.mult, op1=AL.add,
            )
            nc.sync.dma_start(out=out.ap(), in_=acc[:, :])

    nc.finalize()
    return nc


def _host_front(x, w_jeff, w_cc, w_sf0):
    # transpose (T,N,2,C)->(T,N,C,2); synapse filter tau=2; jeff linear;
    # LIF tau=1.5; synapse filter sigmoid(w_sf0); w_cc contract; IF.
    x = np.asarray(x, np.float32).transpose(0, 1, 3, 2)  # (T,N,C,2)
    f = np.zeros_like(x[0])
    ys = np.empty_like(x)
    for t in range(T):
        f = f * np.float32(0.5) + x[t]
        ys[t] = f
    u = np.einsum("tnci,ai->tnca", ys, np.asarray(w_jeff, np.float32)).astype(np.float32)
    inv_tau = np.float32(1.0 / 1.5)
    v = np.zeros(u.shape[1:], np.float32)
    dec0 = (np.float32(1.0) - np.float32(1.0) / (np.float32(1.0) + np.exp(-np.asarray(w_sf0, np.float32))))
    g = np.zeros(u.shape[1:], np.float32)
    wcc = np.asarray(w_cc, np.float32)[0]  # (A,)
    vI = np.zeros((N, C), np.float32)
    s1 = np.empty((T, N, C), np.float32)
    for t in range(T):
        v = v + (u[t] - v) * inv_tau
        s = (v >= 1.0).astype(np.float32)
        v = v * (1.0 - s)
        g = g * dec0 + s
        z = g @ wcc  # (N,C)
        vI = vI + z
        sI = (vI >= 1.0).astype(np.float32)
        vI = vI * (1.0 - sI)
        s1[t] = sI
    return s1  # (T,N,C)


def _prep_in_maps(x, w_jeff, w_cc, w_sf0, W1, w_sf1, W2, w_sf2, W3, w_sf3, W_out, b_out):
    s1 = _host_front(np.asarray(x, np.float32), w_jeff, w_cc, w_sf0)  # (T,N,C)

    def sig(w):
        return 1.0 / (1.0 + np.exp(-float(np.asarray(w))))

    decs = [1.0 - sig(w_sf1), 1.0 - sig(w_sf2), 1.0 - sig(w_sf3)]
    tcol = np.arange(FREE) % T  # n-major: t index of each free column
    maskT = np.empty((128, 3, FREE), np.float32)
    for li, d in enumerate(decs):
        maskT[:, li, :] = np.where(tcol == 0, 0.0, d).astype(np.float32)[None, :]
    cmaskT = np.where(tcol == 0, 0.0, 1.0).astype(np.float32)[None, :]

    base = {"maskT": maskT, "cmaskT": cmaskT}
    rs = []
    for nm, W in (("w1", W1), ("w2", W2), ("w3", W3)):
        wt = np.ascontiguousarray(np.asarray(W, np.float32).T) * np.float32(0.5)
        wh = wt.astype(BF16)
        wl = (wt - wh.astype(np.float32)).astype(BF16)
        base[nm + "h"] = wh
        base[nm + "l"] = wl
        rs.append((wh.astype(np.float32) + wl.astype(np.float32)).sum(axis=0))
    # rowsum corrections as [128, 28] columns: one [128] column per m-tile
    rcat = np.concatenate(rs).astype(np.float32)          # [F1+F2+F3]
    base["rbias"] = np.ascontiguousarray(rcat.reshape(28, 128).T)
    wotq = (np.ascontiguousarray(np.asarray(W_out, np.float32).T) * np.float32(0.5)).astype(BF16)
    base["wot"] = wotq
    b_eff = float(np.asarray(b_out).reshape(-1)[0]) + float(wotq.astype(np.float32).sum())

    in_maps = []
    for c in range(NC_):
        sl = s1[:, c * NB:(c + 1) * NB, :]            # (T, NB, C)
        s1T = np.ascontiguousarray(
            (2.0 * sl.transpose(2, 1, 0).reshape(C, FREE) - 1.0)
        ).astype(BF16)
        d = dict(base)
        d["s1T"] = s1T
        in_maps.append(d)
    return in_maps, b_eff


def kernel(x, w_jeff, w_cc, w_sf0, W1, w_sf1, W2, w_sf2, W3, w_sf3, W_out, b_out):
    in_maps, b_eff = _prep_in_maps(
        x, w_jeff, w_cc, w_sf0, W1, w_sf1, W2, w_sf2, W3, w_sf3, W_out, b_out
    )
    key = ("nc", round(b_eff, 9))
    if key not in _CACHE:
        _CACHE[key] = _build(b_eff)
    nc = _CACHE[key]

    res = run_bass_kernel_spmd(nc, in_maps, core_ids=list(range(NC_)))
    outs = []
    for c in range(NC_):
        o = res.results[c]["out"].reshape(NB, T).T  # (T, NB)
        outs.append(o)
    full = np.concatenate(outs, axis=1)[:, :, None].astype(np.float32)  # (T,N,1)
    return full
